# revision 1
# baseline (speedup 1.0000x reference)
"""Trainium2 Bass kernel for nn_MemoryReader (retrieval_knn).

Math (per batch b):
  mk_h [h,c,n] (c=16, n=THW=8192), qk_h/qe_h [h,c,m] (m=HW=1024)
  logits[h,n,m] = (ms[n]/8) * ( sum_c mk^3*(-qe) + mk*(2*qk*qe) + (-b_sq) )
  aff = softmax over h
  mem[h,c',m] = sum_n mo[h,c',n] * aff[h,n,m]   (c'=128)
  out = concat(mem, qv)

Sharding: 8 cores = 2 batches x 4 THW-chunks (n-chunk 2048/core). Softmax is
over heads -> core-local. Readout partial-sums over n are reduced on host
during the gather (legit unshard of a contraction-sharded axis).

Device kernel per core:
  x  [33, 4*2048]  : per head [mk^3*msn; mk*msn; msn] (msn = ms/8 folded in,
                     row 32 of ones*msn folds the -b_sq term via w row 32)
  w  [33, 4*1024]  : per head [-qe; 2*qk*qe; -b_sq]
  mvt[2048, 512]   : mv chunk transposed (n on partitions for readout matmul)
  -> sim matmul (K=33, fp32r) -> exp (ACT) -> sum/recip/mul (DVE) ->
     readout matmul accumulating over the 16 n-tiles in PSUM -> mem [512,1024]
"""

import sys

sys.path.insert(0, "/opt/trn_rl_repo")

import numpy as np

import concourse.bass as bass
import concourse.tile as tile
from concourse import bacc, mybir
from concourse.bass_utils import run_bass_kernel_spmd

try:
    import ml_dtypes

    _BF16_NP = np.dtype(ml_dtypes.bfloat16)
except ImportError:  # pragma: no cover
    _BF16_NP = None

HEADS, B, CK, CV = 4, 2, 64, 512
T, H, W = 8, 32, 32
THW, HW = T * H * W, H * W          # 8192, 1024
C = CK // HEADS                      # 16
NCHUNK = THW // 4                    # 2048 n per core
NT = NCHUNK // 128                   # 16 n-tiles per core
KDIM = 2 * C + 1                     # 33

F32 = mybir.dt.float32
F32R = mybir.dt.float32r
BF16 = mybir.dt.bfloat16

# ---- tunables -------------------------------------------------------------
USE_F32R_SIM = True      # bitcast sim matmul operands to float32r (4x faster)
USE_F32R_RO = True       # same for readout matmul (only if EW_DT is f32)
EW_DT = BF16             # dtype of e/aff (softmax elementwise) + mvt
RECIP = "approx"         # "approx" (fp32 NR approx) | "plain"
# ---------------------------------------------------------------------------


def _np_dt(dt):
    return _BF16_NP if dt == BF16 else np.float32


def build_bass():
    # Bacc (not plain Bass): its compile()/finalize() pipeline legalizes
    # multi-wait instructions (TRN2 allows 1 wait/inst) via event semaphores.
    nc = bacc.Bacc(None)
    sim_dt = F32R if USE_F32R_SIM else F32
    # float32r must be produced as float32r (verifier: consumer-side bitcast
    # is rejected), so declare the DRAM + SBUF tensors with the dtype the
    # matmul consumes. Bits are identical to f32; numpy side stays float32.
    ro_dt = F32R if (EW_DT == F32 and USE_F32R_RO) else EW_DT
    # xw row-tiled layout: partitions 0-63 hold heads {0,2} (33 real rows,
    # zero-padded to 64), partitions 64-127 hold heads {1,3}. Head pair
    # (2p, 2p+1) runs as two CONCURRENT K=64 matmuls via tile_position
    # (0,0)/(64,0) -- halves sim streaming time on the PE.
    PB = NCHUNK + HW  # per-pair free block: [X 2048 | W 1024]
    xw_d = nc.dram_tensor("xw", [128, 2 * PB], sim_dt, kind="ExternalInput")
    mvt_d = nc.dram_tensor("mvt", [NCHUNK, CV], ro_dt, kind="ExternalInput")
    mem_d = nc.dram_tensor("mem", [CV, HW], F32, kind="ExternalOutput")

    Exp = mybir.ActivationFunctionType.Exp
    Copy = mybir.ActivationFunctionType.Copy

    with tile.TileContext(nc) as tc:
        with (
            tc.tile_pool(name="const", bufs=1) as constp,
            tc.tile_pool(name="simp", bufs=2, space="PSUM") as simp,
            tc.tile_pool(name="memp", bufs=1, space="PSUM") as memp,
            tc.tile_pool(name="work", bufs=6) as work,
            tc.tile_pool(name="outp", bufs=2) as outp,
        ):
            xw_sb = constp.tile([128, 2 * PB], sim_dt)
            # Interleave pair-0/pair-1 chunks (W halves first, then X
            # quarters) so BOTH pairs' first tiles arrive early — the first
            # iteration needs pr0 and pr1 data.
            for wh in range(2):
                for pr in range(2):
                    o = pr * PB + NCHUNK + wh * 512
                    nc.sync.dma_start(
                        out=xw_sb[:, o : o + 512], in_=xw_d[:, o : o + 512]
                    )
            for xh in range(4):
                for pr in range(2):
                    o = pr * PB + xh * (NCHUNK // 4)
                    nc.sync.dma_start(
                        out=xw_sb[:, o : o + NCHUNK // 4],
                        in_=xw_d[:, o : o + NCHUNK // 4],
                    )
            mvt_sb = constp.tile([128, NT * CV], ro_dt)
            for nt in range(NT):
                nc.sync.dma_start(
                    out=mvt_sb[:, nt * CV : (nt + 1) * CV],
                    in_=mvt_d[nt * 128 : (nt + 1) * 128, :],
                )

            # Heater: back-to-back dummy MMs warm the PE (HAM) before the
            # loop. Source is a memset tile (not DMA'd data) so the heater
            # runs DURING the input-DMA wait instead of after it, and the PE
            # is already at K=8/8 when the first sims arrive.
            hsrc = constp.tile([64, 768], BF16)
            nc.vector.memset(hsrc[:], 0.0)
            warm = simp.tile([128, 1024], F32, tag="sim")
            for _ in range(10):
                wmm = nc.tensor.matmul(
                    warm[:, :512],
                    lhsT=hsrc[:, 0:128],
                    rhs=hsrc[:, 128:640],
                    start=True,
                    stop=True,
                    tile_position=(0, 0),
                )
                wmm.ins.bass_priority = -100  # pin to the front of the PE queue

            for mh in range(2):
                mem_ps = memp.tile([128, 4 * 512], F32)
                for nt in range(NT):
                    # --- similarity logits: 4 heads, K=33, N=512 ---
                    simA = simp.tile([128, 1024], F32, tag="sim")
                    simB = simp.tile([128, 1024], F32, tag="sim")
                    for pr in range(2):
                        ps = simA if pr == 0 else simB
                        for half in range(2):
                            base = half * 64
                            nc.tensor.matmul(
                                ps[:, half * 512 : half * 512 + 512],
                                lhsT=xw_sb[base : base + 64,
                                           pr * PB + nt * 128 : pr * PB + nt * 128 + 128],
                                rhs=xw_sb[base : base + 64,
                                          pr * PB + NCHUNK + mh * 512 : pr * PB + NCHUNK + mh * 512 + 512],
                                start=True,
                                stop=True,
                                tile_position=(base, 0),
                            )
                    # --- softmax over heads (no max-sub: |logit| <= ~20) ---
                    e_all = work.tile([128, 2048], EW_DT, tag="e")
                    nc.scalar.activation(e_all[:, :1024], simA[:], Exp)
                    nc.scalar.activation(e_all[:, 1024:], simB[:], Exp)
                    sp = work.tile([128, 1024], EW_DT, tag="sp")
                    nc.vector.tensor_add(sp[:], e_all[:, :1024], e_all[:, 1024:])
                    s_f = work.tile([128, 512], F32, tag="S")
                    nc.gpsimd.tensor_add(s_f[:], sp[:, :512], sp[:, 512:])
                    # custom NR reciprocal writing bf16 directly (out-dtype
                    # conversion happens at the DVE write port) — saves the
                    # separate f32->bf16 cast op.
                    from concourse.dve_ops import (
                        RECIP_APPROX_FAST_CONSTS as _RC,
                        RECIPROCAL_APPROX_FAST as _RF,
                    )
                    r_use = work.tile([128, 512], EW_DT, tag="Rb")
                    nc.vector._custom_dve(
                        _RF,
                        out=r_use[:],
                        in0=s_f[:],
                        s0=_RC["s0"],
                        s1=_RC["s1"],
                        imm2=_RC["imm2"],
                    )
                    aff = work.tile([128, 4 * 512], ro_dt, tag="aff")
                    nc.vector.tensor_mul(
                        aff.rearrange("p (h m) -> p h m", h=4),
                        e_all.rearrange("p (h m) -> p h m", h=4),
                        r_use[:, None, :].to_broadcast((128, 4, 512)),
                    )
                    # --- readout: accumulate over n-tiles in PSUM ---
                    # Deprioritized (higher bass_priority = scheduled later):
                    # readouts only gate the end-of-half flush, while the next
                    # iteration's sims gate the whole softmax chain on ACT/DVE.
                    for h in range(HEADS):
                        ro = nc.tensor.matmul(
                            mem_ps[:, h * 512 : (h + 1) * 512],
                            lhsT=mvt_sb[:, nt * CV + h * 128 : nt * CV + h * 128 + 128],
                            rhs=aff[:, h * 512 : (h + 1) * 512],
                            start=(nt == 0),
                            stop=(nt == NT - 1),
                        )
                        ro.ins.bass_priority = 40
                mem_sb = outp.tile([128, 4 * 512], F32)
                for h in range(HEADS):
                    # per-head copy so each output DMA starts as soon as its
                    # slice is staged (shorter kernel tail)
                    nc.scalar.activation(
                        mem_sb[:, h * 512 : (h + 1) * 512],
                        mem_ps[:, h * 512 : (h + 1) * 512],
                        Copy,
                    )
                    nc.sync.dma_start(
                        out=mem_d[h * 128 : (h + 1) * 128, mh * 512 : (mh + 1) * 512],
                        in_=mem_sb[:, h * 512 : (h + 1) * 512],
                    )
    return nc


def host_decompose(mk, qk, ms, qe, mv):
    """Build the 8 per-core input dicts."""
    mk_f = np.asarray(mk, np.float32).reshape(B, CK, THW)
    mv_f = np.asarray(mv, np.float32).reshape(B, CV, THW)
    ms_f = np.asarray(ms, np.float32).reshape(B, THW)
    qk_h = np.asarray(qk, np.float32).reshape(B, HEADS, C, HW)
    qe_h = np.asarray(qe, np.float32).reshape(B, HEADS, C, HW)

    msn = ms_f / np.float32(np.sqrt(CK))                       # [B, THW]
    mk3 = mk_f * mk_f * mk_f                                   # [B, CK, THW]

    # w [B, 33, h, m]
    w_all = np.empty((B, KDIM, HEADS, HW), np.float32)
    w_all[:, :C] = -np.swapaxes(qe_h, 1, 2)
    w_all[:, C : 2 * C] = np.swapaxes(2.0 * qk_h * qe_h, 1, 2)
    w_all[:, 2 * C] = -np.sum(qe_h * qk_h**3, axis=2)

    # x [B, 33, h, n]
    x_all = np.empty((B, KDIM, HEADS, THW), np.float32)
    mk3_h = mk3.reshape(B, HEADS, C, THW)
    mk_h = mk_f.reshape(B, HEADS, C, THW)
    x_all[:, :C] = np.swapaxes(mk3_h, 1, 2) * msn[:, None, None, :]
    x_all[:, C : 2 * C] = np.swapaxes(mk_h, 1, 2) * msn[:, None, None, :]
    x_all[:, 2 * C] = msn[:, None, :]

    mvt_np = _np_dt(EW_DT)
    PB = NCHUNK + HW
    in_maps = []
    for core in range(8):
        b, j = core // 4, core % 4
        sl = slice(j * NCHUNK, (j + 1) * NCHUNK)
        xw = np.zeros((128, 2 * PB), np.float32)
        for pr in range(2):
            for half in range(2):
                h = 2 * pr + half
                r0 = half * 64
                xw[r0 : r0 + KDIM, pr * PB : pr * PB + NCHUNK] = x_all[b, :, h, sl]
                xw[r0 : r0 + KDIM, pr * PB + NCHUNK : (pr + 1) * PB] = w_all[b, :, h]
        mvt = np.ascontiguousarray(mv_f[b, :, sl].T).astype(mvt_np)
        in_maps.append({"xw": xw, "mvt": mvt})
    return in_maps


_NC_CACHE = None


def _get_nc():
    global _NC_CACHE
    if _NC_CACHE is None:
        nc = build_bass()
        if not nc.is_finalized():
            nc.finalize()  # Bacc compile: wait legalization etc.
        _NC_CACHE = nc
    return _NC_CACHE


def kernel(mk, qk, ms, qe, mv, qv, _trace=False, _trace_kwargs=None):
    in_maps = host_decompose(mk, qk, ms, qe, mv)
    nc = _get_nc()
    res = run_bass_kernel_spmd(
        nc, in_maps, list(range(8)), trace=_trace, **(_trace_kwargs or {})
    )
    mem = np.zeros((B, CV, HW), np.float32)
    for core in range(8):
        mem[core // 4] += res.results[core]["mem"]
    out = np.concatenate(
        [mem.reshape(B, CV, H, W), np.asarray(qv, np.float32).reshape(B, CV, H, W)],
        axis=1,
    )
    if _trace:
        return out, res
    return out



# revision 3
# speedup vs baseline: 1.2512x; 1.2512x over previous
"""Trainium2 Bass kernel for nn_MemoryReader (retrieval_knn).

Math (per batch b):
  mk_h [h,c,n] (c=16, n=THW=8192), qk_h/qe_h [h,c,m] (m=HW=1024)
  logits[h,n,m] = (ms[n]/8) * ( sum_c mk^3*(-qe) + mk*(2*qk*qe) + (-b_sq) )
  aff = softmax over h
  mem[h,c',m] = sum_n mo[h,c',n] * aff[h,n,m]   (c'=128)
  out = concat(mem, qv)

Sharding: 8 cores = 2 batches x 4 THW-chunks (n-chunk 2048/core). Softmax is
over heads -> core-local. Readout partial-sums over n are reduced on host
during the gather (legit unshard of a contraction-sharded axis).

Device kernel per core:
  x  [33, 4*2048]  : per head [mk^3*msn; mk*msn; msn] (msn = ms/8 folded in,
                     row 32 of ones*msn folds the -b_sq term via w row 32)
  w  [33, 4*1024]  : per head [-qe; 2*qk*qe; -b_sq]
  mvt[2048, 512]   : mv chunk transposed (n on partitions for readout matmul)
  -> sim matmul (K=33, bf16) -> exp (ACT) -> sum (DVE) / sum (GPS) / recip
     (DVE custom NR) / mul (DVE) -> readout matmul accumulating over the 16
     n-tiles in PSUM -> mem [512,1024]

Pipeline: readout matmuls are emitted LAG=2 iterations behind the softmax
chain that produces their aff operand, so the PE never stalls on the
ACT->DVE->GPS->DVE chain latency (it runs the next iterations' sims
instead); this also keeps PE activity dense enough for HAM to hold the
2.4GHz clock (K=8/8) instead of oscillating to 1.2GHz.
"""

import sys

sys.path.insert(0, "/opt/trn_rl_repo")

import numpy as np

import concourse.bass as bass
import concourse.tile as tile
from concourse import bacc, mybir
from concourse.bass_utils import run_bass_kernel_spmd

try:
    import ml_dtypes

    _BF16_NP = np.dtype(ml_dtypes.bfloat16)
except ImportError:  # pragma: no cover
    _BF16_NP = None

HEADS, B, CK, CV = 4, 2, 64, 512
T, H, W = 8, 32, 32
THW, HW = T * H * W, H * W          # 8192, 1024
C = CK // HEADS                      # 16
NCHUNK = THW // 4                    # 2048 n per core
NT = NCHUNK // 128                   # 16 n-tiles per core
KDIM = 2 * C + 1                     # 33
NITER = 2 * NT                       # (mh, nt) flat iteration count
LAG = 2                              # readout lag (iterations)

F32 = mybir.dt.float32
BF16 = mybir.dt.bfloat16

SIM_DT = BF16            # x/w dtype (bf16: 1 cyc/col at every PE p-state)
EW_DT = BF16             # dtype of e/aff (softmax elementwise) + mvt


def _np_dt(dt):
    return _BF16_NP if dt == BF16 else np.float32


def build_bass():
    # Bacc (not plain Bass): its compile()/finalize() pipeline legalizes
    # multi-wait instructions (TRN2 allows 1 wait/inst) via event semaphores.
    nc = bacc.Bacc(None)
    # xw row-tiled layout: partitions 0-63 hold heads {0,2} (33 real rows,
    # zero-padded to 64), partitions 64-127 hold heads {1,3}. Head pair
    # (2p, 2p+1) runs as two CONCURRENT K=64 matmuls via tile_position
    # (0,0)/(64,0) -- halves sim streaming time on the PE.
    PB = NCHUNK + HW  # per-pair free block: [X 2048 | W 1024]
    xw_d = nc.dram_tensor("xw", [128, 2 * PB], SIM_DT, kind="ExternalInput")
    mvt_d = nc.dram_tensor("mvt", [NCHUNK, CV], EW_DT, kind="ExternalInput")
    mem_d = nc.dram_tensor("mem", [CV, HW], F32, kind="ExternalOutput")

    Exp = mybir.ActivationFunctionType.Exp
    Copy = mybir.ActivationFunctionType.Copy

    from concourse.dve_ops import (
        RECIP_APPROX_FAST_CONSTS as _RC,
        RECIPROCAL_APPROX_FAST as _RF,
    )

    with tile.TileContext(nc) as tc:
        with (
            tc.tile_pool(name="const", bufs=1) as constp,
            tc.tile_pool(name="simp", bufs=2, space="PSUM") as simp,
            tc.tile_pool(name="memp", bufs=1, space="PSUM") as memp,
            tc.tile_pool(name="work", bufs=6) as work,
            tc.tile_pool(name="outp", bufs=2) as outp,
        ):
            xw_sb = constp.tile([128, 2 * PB], SIM_DT)
            # Interleave pair-0/pair-1 chunks (W halves first, then X
            # quarters) so BOTH pairs' first tiles arrive early — the first
            # iteration needs pr0 and pr1 data.
            for wh in range(2):
                for pr in range(2):
                    o = pr * PB + NCHUNK + wh * 512
                    nc.sync.dma_start(
                        out=xw_sb[:, o : o + 512], in_=xw_d[:, o : o + 512]
                    )
            for xh in range(4):
                for pr in range(2):
                    o = pr * PB + xh * (NCHUNK // 4)
                    nc.sync.dma_start(
                        out=xw_sb[:, o : o + NCHUNK // 4],
                        in_=xw_d[:, o : o + NCHUNK // 4],
                    )
            mvt_sb = constp.tile([128, NT * CV], EW_DT)
            for nt in range(NT):
                nc.sync.dma_start(
                    out=mvt_sb[:, nt * CV : (nt + 1) * CV],
                    in_=mvt_d[nt * 128 : (nt + 1) * 128, :],
                )

            # Heater: back-to-back dummy MMs warm the PE (HAM) before the
            # loop. Source is a memset tile (not DMA'd data) so the heater
            # runs DURING the input-DMA wait instead of after it, and the PE
            # is already at K=8/8 when the first sims arrive.
            hsrc = constp.tile([64, 768], BF16)
            nc.vector.memset(hsrc[:], 0.0)
            warm = simp.tile([128, 1024], F32, tag="sim")
            for _ in range(10):
                wmm = nc.tensor.matmul(
                    warm[:, :512],
                    lhsT=hsrc[:, 0:128],
                    rhs=hsrc[:, 128:640],
                    start=True,
                    stop=True,
                    tile_position=(0, 0),
                )
                wmm.ins.bass_priority = -100  # pin to the front of the PE queue

            aff_tiles = {}
            mem_tiles = {}
            for it in range(NITER + LAG):
                if it < NITER:
                    mh, nt = divmod(it, NT)
                    # --- similarity logits: 4 heads, K=33, N=512 ---
                    simA = simp.tile([128, 1024], F32, tag="sim")
                    simB = simp.tile([128, 1024], F32, tag="sim")
                    for pr in range(2):
                        ps = simA if pr == 0 else simB
                        for half in range(2):
                            base = half * 64
                            nc.tensor.matmul(
                                ps[:, half * 512 : half * 512 + 512],
                                lhsT=xw_sb[base : base + 64,
                                           pr * PB + nt * 128 : pr * PB + nt * 128 + 128],
                                rhs=xw_sb[base : base + 64,
                                          pr * PB + NCHUNK + mh * 512 : pr * PB + NCHUNK + mh * 512 + 512],
                                start=True,
                                stop=True,
                                tile_position=(base, 0),
                            )
                    # --- softmax over heads (no max-sub: |logit| <= ~25) ---
                    e_all = work.tile([128, 2048], EW_DT, tag="e")
                    nc.scalar.activation(e_all[:, :1024], simA[:], Exp)
                    nc.scalar.activation(e_all[:, 1024:], simB[:], Exp)
                    sp = work.tile([128, 1024], EW_DT, tag="sp")
                    nc.vector.tensor_add(sp[:], e_all[:, :1024], e_all[:, 1024:])
                    s_f = work.tile([128, 512], F32, tag="S")
                    nc.gpsimd.tensor_add(s_f[:], sp[:, :512], sp[:, 512:])
                    # custom NR reciprocal writing bf16 directly (out-dtype
                    # conversion happens at the DVE write port) — saves the
                    # separate f32->bf16 cast op.
                    r_use = work.tile([128, 512], EW_DT, tag="Rb")
                    nc.vector._custom_dve(
                        _RF,
                        out=r_use[:],
                        in0=s_f[:],
                        s0=_RC["s0"],
                        s1=_RC["s1"],
                        imm2=_RC["imm2"],
                    )
                    aff = work.tile([128, 4 * 512], EW_DT, tag="aff")
                    nc.vector.tensor_mul(
                        aff.rearrange("p (h m) -> p h m", h=4),
                        e_all.rearrange("p (h m) -> p h m", h=4),
                        r_use[:, None, :].to_broadcast((128, 4, 512)),
                    )
                    aff_tiles[it] = aff
                # --- readout: LAG iterations behind, accumulate over nt ---
                ro = it - LAG
                if 0 <= ro < NITER:
                    mh_r, nt_r = divmod(ro, NT)
                    if nt_r == 0:
                        mem_tiles[mh_r] = memp.tile(
                            [128, 4 * 512], F32, tag="mem", name=f"mem_ps{mh_r}"
                        )
                    mem_ps = mem_tiles[mh_r]
                    aff_r = aff_tiles.pop(ro)
                    for h in range(HEADS):
                        nc.tensor.matmul(
                            mem_ps[:, h * 512 : (h + 1) * 512],
                            lhsT=mvt_sb[:, nt_r * CV + h * 128 : nt_r * CV + h * 128 + 128],
                            rhs=aff_r[:, h * 512 : (h + 1) * 512],
                            start=(nt_r == 0),
                            stop=(nt_r == NT - 1),
                        )
                    if nt_r == NT - 1:
                        mem_sb = outp.tile([128, 4 * 512], F32)
                        for h in range(HEADS):
                            # per-head copy, alternating ACT/DVE, so each
                            # output DMA starts as soon as its slice is
                            # staged (shorter kernel tail)
                            dst = mem_sb[:, h * 512 : (h + 1) * 512]
                            src = mem_ps[:, h * 512 : (h + 1) * 512]
                            if h % 2 == 0:
                                nc.scalar.activation(dst, src, Copy)
                            else:
                                nc.vector.tensor_copy(dst, src)
                            nc.sync.dma_start(
                                out=mem_d[h * 128 : (h + 1) * 128,
                                          mh_r * 512 : (mh_r + 1) * 512],
                                in_=mem_sb[:, h * 512 : (h + 1) * 512],
                            )
    return nc


def host_decompose(mk, qk, ms, qe, mv):
    """Build the 8 per-core input dicts."""
    mk_f = np.asarray(mk, np.float32).reshape(B, CK, THW)
    mv_f = np.asarray(mv, np.float32).reshape(B, CV, THW)
    ms_f = np.asarray(ms, np.float32).reshape(B, THW)
    qk_h = np.asarray(qk, np.float32).reshape(B, HEADS, C, HW)
    qe_h = np.asarray(qe, np.float32).reshape(B, HEADS, C, HW)

    msn = ms_f / np.float32(np.sqrt(CK))                       # [B, THW]
    mk3 = mk_f * mk_f * mk_f                                   # [B, CK, THW]

    # w [B, 33, h, m]
    w_all = np.empty((B, KDIM, HEADS, HW), np.float32)
    w_all[:, :C] = -np.swapaxes(qe_h, 1, 2)
    w_all[:, C : 2 * C] = np.swapaxes(2.0 * qk_h * qe_h, 1, 2)
    w_all[:, 2 * C] = -np.sum(qe_h * qk_h**3, axis=2)

    # x [B, 33, h, n]
    x_all = np.empty((B, KDIM, HEADS, THW), np.float32)
    mk3_h = mk3.reshape(B, HEADS, C, THW)
    mk_h = mk_f.reshape(B, HEADS, C, THW)
    x_all[:, :C] = np.swapaxes(mk3_h, 1, 2) * msn[:, None, None, :]
    x_all[:, C : 2 * C] = np.swapaxes(mk_h, 1, 2) * msn[:, None, None, :]
    x_all[:, 2 * C] = msn[:, None, :]

    sim_np = _np_dt(SIM_DT)
    mvt_np = _np_dt(EW_DT)
    PB = NCHUNK + HW
    in_maps = []
    for core in range(8):
        b, j = core // 4, core % 4
        sl = slice(j * NCHUNK, (j + 1) * NCHUNK)
        xw = np.zeros((128, 2 * PB), sim_np)
        for pr in range(2):
            for half in range(2):
                h = 2 * pr + half
                r0 = half * 64
                xw[r0 : r0 + KDIM, pr * PB : pr * PB + NCHUNK] = x_all[b, :, h, sl].astype(sim_np)
                xw[r0 : r0 + KDIM, pr * PB + NCHUNK : (pr + 1) * PB] = w_all[b, :, h].astype(sim_np)
        mvt = np.ascontiguousarray(mv_f[b, :, sl].T).astype(mvt_np)
        in_maps.append({"xw": xw, "mvt": mvt})
    return in_maps


_NC_CACHE = None


def _get_nc():
    global _NC_CACHE
    if _NC_CACHE is None:
        nc = build_bass()
        if not nc.is_finalized():
            nc.finalize()  # Bacc compile: wait legalization etc.
        _NC_CACHE = nc
    return _NC_CACHE


def kernel(mk, qk, ms, qe, mv, qv, _trace=False, _trace_kwargs=None):
    in_maps = host_decompose(mk, qk, ms, qe, mv)
    nc = _get_nc()
    res = run_bass_kernel_spmd(
        nc, in_maps, list(range(8)), trace=_trace, **(_trace_kwargs or {})
    )
    mem = np.zeros((B, CV, HW), np.float32)
    for core in range(8):
        mem[core // 4] += res.results[core]["mem"]
    out = np.concatenate(
        [mem.reshape(B, CV, H, W), np.asarray(qv, np.float32).reshape(B, CV, H, W)],
        axis=1,
    )
    if _trace:
        return out, res
    return out
